# revision 2
# baseline (speedup 1.0000x reference)
"""DiceLoss kernel v3 for Trainium2 (8 NeuronCores, SPMD data-parallel).

Problem: input [2,4,128,160,160] f32 logits, target [2,128,160,160] int
  pred = argmax(input, axis=1); for classes 1..3:
  inter_c = |pred==c & tgt==c|, union_c = |pred==c| + |tgt==c| - inter_c
  loss = 1 - mean_{b,c}( (inter+eps)/(union+eps) )

Sharding: flatten spatial dims; each core gets a contiguous 1/8 slice
(S=409,600 voxels) of both batches.

v3 design (empirical cost model from ubench):
  - host casts x to bf16 during sharding -> HBM traffic halves (DMA floor
    ~21us). bf16 argmax ties inflate counts ~0.2%, fine at 2e-2 tol.
  - DVE (2x ops only; STT/reduce are 1x -> avoided on big tensors):
      max tree (3x TT), pm_c = is_equal(x_c, m) (3x TT),
      tm_1 = (t==1) via TS with accum_out (2x even on u8).
  - ACT: tm_2, tm_3 via Square(t-c) + Relu(1-sq) with accum_out.
  - PE (otherwise idle): inter via pm^T@tm 128x128 diag-accumulated in
    PSUM; |pred==c| via ones^T@pm column sums folded into [1,512] PSUM.
  - drains: diag*I dot via STT+accum (tiny), colsum sums via TS+accum
    (tiny), final ones^T@acc f32 matmul -> out is one [1,32] f32 row
    (128 B) per core; host sums 8 rows and finishes the scalar math.
"""

import sys

sys.path.insert(0, "/opt/trn_rl_repo")

import numpy as np

B = 2
C = 4
N_SP = 128 * 160 * 160
N_CORES = 8
S = N_SP // N_CORES           # 409,600 voxels per core per batch
P = 128
SF = S // P                   # 3200 free elems per partition
CHUNKS = [(0, 512), (512, 1024), (1536, 1664)]
NCH = len(CHUNKS)
NACC = B * NCH * 3 + B * 3    # 18 tm cols + 6 inter cols
NOUT = 32                     # 24 (matmul) + 6 (pm colsums) padded
EPS = 1e-08

_CACHE = {}


def _build_bass():
    import concourse.bass as bass
    import concourse.tile as tile
    from concourse import bacc, mybir
    from contextlib import ExitStack

    f32 = mybir.dt.float32
    bf16 = mybir.dt.bfloat16
    u8 = mybir.dt.uint8
    Alu = mybir.AluOpType
    Act = mybir.ActivationFunctionType

    nc = bacc.Bacc()

    x = nc.declare_dram_parameter("x", [B, C, S], bf16, isOutput=False)
    t = nc.declare_dram_parameter("t", [B, S], u8, isOutput=False)
    ident_d = nc.declare_dram_parameter("ident", [P, P], bf16, isOutput=False)
    cnt_d = nc.declare_dram_parameter("counts", [1, NOUT], f32, isOutput=True)

    def tm_col(b, j, ci):
        return (b * NCH + j) * 3 + ci

    def it_col(b, ci):
        return B * NCH * 3 + b * 3 + ci

    with ExitStack() as ctx:
        tc = ctx.enter_context(tile.TileContext(nc))
        const_pool = ctx.enter_context(tc.tile_pool(name="const", bufs=1))
        xpool = ctx.enter_context(tc.tile_pool(name="xp", bufs=3))
        tpool = ctx.enter_context(tc.tile_pool(name="tp", bufs=3))
        mpool = ctx.enter_context(tc.tile_pool(name="mp", bufs=2))
        kpool = ctx.enter_context(tc.tile_pool(name="kp", bufs=2))
        dpool = ctx.enter_context(tc.tile_pool(name="dp", bufs=2))
        # one PSUM bank per diag tile: matmul start=True resets has_written
        # at bank granularity, so accumulation groups must not share a bank
        psd = ctx.enter_context(tc.tile_pool(name="psd", bufs=1, space="PSUM"))
        psc = ctx.enter_context(tc.tile_pool(name="psc", bufs=1, space="PSUM"))
        ps2 = ctx.enter_context(tc.tile_pool(name="ps2", bufs=1, space="PSUM"))

        acc = const_pool.tile([P, NACC], f32)
        sb = const_pool.tile([1, NOUT], f32, tag="sb", name="sb")
        ones_f = const_pool.tile([P, 1], f32, tag="onesf", name="onesf")
        nc.vector.memset(ones_f[:], 1.0)
        ones_b = const_pool.tile([P, 1], bf16, tag="onesb", name="onesb")
        nc.vector.memset(ones_b[:], 1.0)
        ident = const_pool.tile([P, P], bf16, tag="ident", name="ident")
        nc.sync.dma_start(out=ident[:], in_=ident_d[:, :])
        neg_c = []
        for c in (2, 3):
            bias_t = const_pool.tile([P, 1], f32, tag=f"bias{c}", name=f"bias{c}")
            nc.vector.memset(bias_t[:], -float(c))
            neg_c.append(bias_t)

        for b in range(B):
            xsrc = x[b].rearrange("c (p f) -> c p f", p=P)
            tsrc = t[b].rearrange("(p f) -> p f", p=P)
            ps_diag = [
                psd.tile([P, P], f32, tag=f"diag{ci}", name=f"psdiag{ci}")[:, :]
                for ci in range(3)
            ]
            ps_cnt = [
                psc.tile([1, 512], f32, tag=f"cnt{ci}", name=f"pscnt{ci}")
                for ci in range(3)
            ]

            for j, (f0, F) in enumerate(CHUNKS):
                xts = []
                for ci in range(C):
                    xc = xpool.tile([P, F], bf16, tag=f"x{ci}", name=f"x{ci}")
                    eng = nc.sync if ci < 2 else nc.scalar
                    eng.dma_start(out=xc[:], in_=xsrc[ci, :, f0 : f0 + F])
                    xts.append(xc)
                tt = tpool.tile([P, F], u8, tag="tt")
                nc.gpsimd.dma_start(out=tt[:], in_=tsrc[:, f0 : f0 + F])

                m01 = mpool.tile([P, F], bf16, tag="m01")
                nc.vector.tensor_tensor(m01[:], xts[0][:], xts[1][:], op=Alu.max)
                m23 = mpool.tile([P, F], bf16, tag="m23")
                nc.vector.tensor_tensor(m23[:], xts[2][:], xts[3][:], op=Alu.max)
                m = mpool.tile([P, F], bf16, tag="m")
                nc.vector.tensor_tensor(m[:], m01[:], m23[:], op=Alu.max)

                pms, tms = [], []
                for ci in range(3):
                    pm = kpool.tile([P, F], bf16, tag=f"pm{ci}", name=f"pm{ci}")
                    nc.vector.tensor_tensor(
                        pm[:], xts[ci + 1][:], m[:], op=Alu.is_equal
                    )
                    pms.append(pm)
                # tm_1 on DVE (TS is 2x, with free accum)
                tm1 = kpool.tile([P, F], bf16, tag="tm0", name="tm0")
                cc = tm_col(b, j, 0)
                nc.vector.tensor_scalar(
                    tm1[:], tt[:], 1.0, 0.0, op0=Alu.is_equal, op1=Alu.add,
                    accum_out=acc[:, cc : cc + 1],
                )
                tms.append(tm1)
                # tm_2, tm_3 on ACT
                for k, c in enumerate((2, 3)):
                    sq = kpool.tile([P, F], bf16, tag=f"sq{c}", name=f"sq{c}")
                    nc.scalar.activation(
                        sq[:], tt[:], Act.Square, bias=neg_c[k][:], scale=1.0
                    )
                    tm = kpool.tile([P, F], bf16, tag=f"tm{c}", name=f"tm{c}")
                    cc = tm_col(b, j, k + 1)
                    nc.scalar.activation(
                        tm[:], sq[:], Act.Relu, bias=1.0, scale=-1.0,
                        accum_out=acc[:, cc : cc + 1],
                    )
                    tms.append(tm)

                last_j = j == NCH - 1
                ns128 = F // 128
                for ci in range(3):
                    for si in range(ns128):
                        sl = slice(si * 128, (si + 1) * 128)
                        nc.tensor.matmul(
                            ps_diag[ci],
                            pms[ci][:, sl],
                            tms[ci][:, sl],
                            start=(j == 0 and si == 0),
                            stop=(last_j and si == ns128 - 1),
                        )
                # pm colsums (ones stationary), folded mod 512
                offs = []
                off = 0
                while off < F:
                    offs.append((off, min(512, F - off)))
                    off += 512
                for ci in range(3):
                    for oi, (off, ns) in enumerate(offs):
                        oc = (f0 + off) % 512
                        nc.tensor.matmul(
                            ps_cnt[ci][0:1, oc : oc + ns],
                            ones_b[:],
                            pms[ci][:, off : off + ns],
                            start=(j == 0 and oi == 0),
                            stop=(last_j and oi == len(offs) - 1),
                        )

            # drains (tiny): diag . I -> acc col ; colsum sum -> sb col
            for ci in range(3):
                dsc = dpool.tile([P, P], f32, tag=f"dsc{ci}", name=f"dsc{ci}")
                cc = it_col(b, ci)
                nc.vector.scalar_tensor_tensor(
                    dsc[:], ps_diag[ci], 0.0, ident[:],
                    op0=Alu.add, op1=Alu.mult,
                    accum_out=acc[:, cc : cc + 1],
                )
                csc = dpool.tile([1, 512], f32, tag=f"csc{ci}", name=f"csc{ci}")
                oc = 24 + b * 3 + ci
                nc.vector.tensor_scalar(
                    csc[:], ps_cnt[ci][:], 0.0, 0.0, op0=Alu.add, op1=Alu.add,
                    accum_out=sb[0:1, oc : oc + 1],
                )

        psf = ps2.tile([1, NACC], f32, name="psf")
        nc.tensor.matmul(psf[:, :], ones_f[:], acc[:], start=True, stop=True)
        nc.scalar.copy(sb[0:1, 0:NACC], psf[:])
        nc.sync.dma_start(out=cnt_d[:, :], in_=sb[:])

    nc.compile()
    return nc


def _get_nc():
    if "nc" not in _CACHE:
        _CACHE["nc"] = _build_bass()
    return _CACHE["nc"]


def _shard_inputs(input, target):
    import ml_dtypes

    inp = np.asarray(input).reshape(B, C, N_SP).astype(ml_dtypes.bfloat16)
    tgt = np.asarray(target).reshape(B, N_SP)
    ident = np.eye(P, dtype=ml_dtypes.bfloat16)
    in_maps = []
    for r in range(N_CORES):
        xr = np.ascontiguousarray(inp[:, :, r * S : (r + 1) * S])
        tr = np.ascontiguousarray(tgt[:, r * S : (r + 1) * S].astype(np.uint8))
        in_maps.append({"x": xr, "t": tr, "ident": ident})
    return in_maps


def _finish(results):
    pred_cnt = np.zeros((B, 3), np.float64)
    tgt_cnt = np.zeros((B, 3), np.float64)
    inter = np.zeros((B, 3), np.float64)
    for res in results:
        row = np.asarray(res["counts"], np.float64)[0]
        tm = row[: B * NCH * 3].reshape(B, NCH, 3)
        tgt_cnt += tm.sum(axis=1)
        inter += row[B * NCH * 3 : B * NCH * 3 + B * 3].reshape(B, 3)
        pred_cnt += row[24 : 24 + B * 3].reshape(B, 3)
    union = pred_cnt + tgt_cnt - inter
    dice = (inter + EPS) / (union + EPS)
    return np.float32(1.0 - dice.mean())


def kernel(input, target):
    from concourse.bass_utils import run_bass_kernel_spmd

    nc = _get_nc()
    in_maps = _shard_inputs(input, target)
    out = run_bass_kernel_spmd(nc, in_maps, core_ids=list(range(N_CORES)))
    return _finish(out.results)


if __name__ == "__main__":
    rng = np.random.default_rng(0)
    inp = rng.standard_normal((B, C, 128, 160, 160), dtype=np.float32)
    tgt = rng.integers(0, C, size=(B, 128, 160, 160)).astype(np.int32)

    got = kernel(input=inp, target=tgt)

    import ml_dtypes
    xb = inp.astype(ml_dtypes.bfloat16).astype(np.float32)
    pred_m = xb.max(axis=1, keepdims=True)
    pmf = (xb == pred_m).reshape(B, C, -1)
    tg = tgt.reshape(B, -1)
    dice = np.zeros((B, 3))
    for b in range(B):
        for ci, c in enumerate((1, 2, 3)):
            pm = pmf[b, c]
            tm = tg[b] == c
            i = np.sum(pm & tm)
            u = pm.sum() + tm.sum() - i
            dice[b, ci] = (i + EPS) / (u + EPS)
    want = np.float32(1.0 - dice.mean())
    print("kernel:", got, "bf16-ref:", want, "relerr:", abs(got - want) / abs(want))
